# revision 13
# baseline (speedup 1.0000x reference)
"""Trainium2 Bass kernel for per-sample channel-modulated 3x3 conv (CoModConv).

Math (matches the reference nn.Module):
    s = lrelu(lrelu(lrelu(y @ w0.T + b0) @ w1.T + b1) @ w2.T + b2)   # (B, C_in)
    out = conv3x3(x * s[:, :, None, None], conv_w, pad=1)            # (B, C_out, H, W)

Strategy: data-parallel over batch, 2 samples per NeuronCore (8 cores), with a
1-D Winograd F(4,3) transform along H (direct 3-tap conv along W), all in fp16:
  - Host precomputes U0 = G @ conv_w (transform of the shared weight along kh);
    the per-sample channel scale s folds into U on device (one tensor_scalar
    per (sample, ci-tile)), so modulation is free.
  - Device builds V = B^T d (6 transformed row-planes per input tile of 4 rows)
    on the vector engine (fp16, 2x mode), with the constant-scale steps on the
    scalar engine.
  - The conv becomes, per (sample, co_t, 8-i-tile chunk), 36 accumulating
    128x128x512 fp16 matmuls (ci_t x kw x u) into 6 PSUM banks -- 288 matmuls
    per core vs 576 for direct conv (1.5x fewer after transform overhead;
    2.25x fewer MACs land in 6/9 of the direct kernel taps).
  - Inverse transform A^T m (6 -> 4 output rows) runs on scalar-engine drains
    (PSUM fp32 -> fp16) + 13 small vector ops per chunk, writing interleaved
    output rows; outputs DMA out in fp16 and are upcast on host.
fp16 (not bf16) keeps the Winograd transform numerics comfortably inside the
tolerance: measured rel err ~4.6e-3 vs ~4.3e-2 for bf16.
"""

import numpy as np

B, D_CAT, C_IN, C_OUT, K, H, W = 16, 512, 256, 256, 3, 64, 64
NCORES = 8
BL = B // NCORES          # samples per core (2)
CIT = C_IN // 128         # ci tiles (2)
COT = C_OUT // 128        # co tiles (2)
GH = H + 2                # padded grid rows (66)
GW = W + 2                # padded grid cols (66)
UD = 6                    # F(4,3) transform length
IT = 16                   # winograd i-tiles along H (4 output rows each)
CHI = 8                   # i-tiles per PSUM chunk (8*64 = 512 cols)
NCH = IT // CHI           # chunks per (sample, co_t) (2)
UBF = UD * K * 128        # Ub columns per co_t block (2304)

# packed MLP-param column offsets (per partition). Weights and y ship in bf16
# (pp1 = y + w0, pp2 = w1, pp3 = w2, ordered by first use); biases in fp32.
_PY = 0                       # y^T:   4 k-tiles x BL
_PW0 = _PY + 4 * BL           # w0^T:  4 k-tiles x 256
_P1TOT = _PW0 + 4 * C_IN
_P2TOT = 2 * C_IN             # w1^T
_P3TOT = 2 * C_IN             # w2^T
_NBIAS = 3 * CIT              # b0, b1, b2 per ci-tile (fp32)

# F(4,3) weight transform (G), with the sign of row u=1 folded in because the
# device computes V[1] = 4(d1+d2) - (d3+d4) = -B^T[1] d.
_G = np.array(
    [
        [1 / 4, 0, 0],
        [-1 / 6, -1 / 6, -1 / 6],
        [-1 / 6, 1 / 6, -1 / 6],
        [1 / 24, 1 / 12, 1 / 6],
        [1 / 24, -1 / 12, 1 / 6],
        [0, 0, 1],
    ],
    dtype=np.float64,
) * np.array([1, -1, 1, 1, 1, 1], dtype=np.float64)[:, None]

_COMPILED = None


def _build():
    import concourse.mybir as mybir
    import concourse.tile as tile
    from concourse import bacc

    bf16 = mybir.dt.bfloat16
    f16 = mybir.dt.float16
    f32 = mybir.dt.float32
    Prelu = mybir.ActivationFunctionType.Prelu
    ADD = mybir.AluOpType.add
    SUB = mybir.AluOpType.subtract

    nc = bacc.Bacc("TRN2", target_bir_lowering=False, debug=False, num_devices=NCORES)

    pp1_in = nc.declare_dram_parameter("pp1", [128, _P1TOT], bf16, isOutput=False)
    pp2_in = nc.declare_dram_parameter("pp2", [128, _P2TOT], bf16, isOutput=False)
    pp3_in = nc.declare_dram_parameter("pp3", [128, _P3TOT], bf16, isOutput=False)
    bias_in = nc.declare_dram_parameter("bias", [128, _NBIAS], f32, isOutput=False)
    u0_in = nc.declare_dram_parameter("u0", [CIT, COT, 128, UBF], f16, isOutput=False)
    xg_in = nc.declare_dram_parameter("xg", [BL, CIT, 128, GH * GW], f16, isOutput=False)
    out_ext = nc.declare_dram_parameter("out", [BL, COT, 128, H * W], f16, isOutput=True)

    with tile.TileContext(nc) as tc:
        with (
            tc.tile_pool(name="const", bufs=1) as cpool,
            tc.tile_pool(name="grid", bufs=2) as gpool,
            tc.tile_pool(name="vpool", bufs=4) as vpool,
            tc.tile_pool(name="tmp", bufs=2) as tpool,
            tc.tile_pool(name="u0p", bufs=2) as upool,
            tc.tile_pool(name="minv", bufs=2) as mpool,
            tc.tile_pool(name="oout", bufs=2) as opool,
            tc.tile_pool(name="cpsum", bufs=8, space="PSUM") as cpsum,
        ):
            MAX = mybir.AluOpType.max
            grids = {}
            u0_sbs = {}

            def load_grid(b, ci_t):
                t = gpool.tile([128, GH * GW], f16, name=f"g{b}{ci_t}", tag="g")
                nc.gpsimd.dma_start(t[:], xg_in[b, ci_t])
                grids[(b, ci_t)] = t[:].rearrange("p (h w) -> p h w", w=GW)

            def load_u0(ci_t, co_t):
                t = upool.tile([128, UBF], f16, name=f"u0_{ci_t}{co_t}", tag="u0")
                nc.gpsimd.dma_start(t[:], u0_in[ci_t, co_t])
                u0_sbs[(ci_t, co_t)] = t

            # ---- DMAs, ordered so the first-chunk dependencies land first.
            # x grids and U0 go via the gpsimd SWDGE queue, params via HWDGE;
            # the sim serializes all DMA on one device, so order IS latency.
            load_grid(0, 0)
            pp1_sb = cpool.tile([128, _P1TOT], bf16)
            nc.sync.dma_start(pp1_sb[:], pp1_in[:])
            bias_sb = cpool.tile([128, _NBIAS], f32)
            nc.sync.dma_start(bias_sb[:], bias_in[:])
            load_u0(0, 0)
            pp2_sb = cpool.tile([128, _P2TOT], bf16)
            nc.sync.dma_start(pp2_sb[:], pp2_in[:])
            pp3_sb = cpool.tile([128, _P3TOT], bf16)
            nc.sync.dma_start(pp3_sb[:], pp3_in[:])
            load_u0(0, 1)
            load_grid(0, 1)
            load_u0(1, 0)
            load_u0(1, 1)

            # ---- style MLP. Matmuls on PE; bias + leaky-relu run on the
            # (otherwise idle) gpsimd engine so the scalar engine's in-order
            # queue stays free for V-transform scale ops. ----
            def mlp_layer(rhs_of_kt, kts, w_sb, w_base, bias_col, out_sb):
                for ct in range(CIT):
                    mps = cpsum.tile([128, 512], f32, name=f"mlp_{w_base}_{ct}", tag="ps")
                    for kt in range(kts):
                        nc.tensor.matmul(
                            mps[:, :BL],
                            w_sb[:, w_base + kt * C_IN + ct * 128 :][:, :128],
                            rhs_of_kt(kt),
                            start=(kt == 0),
                            stop=(kt == kts - 1),
                        )
                    za = tpool.tile([128, BL], f32, name=f"za_{w_base}_{ct}", tag="za")
                    zl = tpool.tile([128, BL], f32, name=f"zl_{w_base}_{ct}", tag="zl")
                    nc.gpsimd.tensor_scalar_add(
                        za[:], mps[:, :BL], bias_sb[:, bias_col + ct : bias_col + ct + 1]
                    )
                    nc.gpsimd.tensor_scalar_mul(zl[:], za[:], 0.01)
                    nc.gpsimd.tensor_tensor(
                        out_sb[:, ct * BL : (ct + 1) * BL], za[:], zl[:], MAX
                    )

            s0_sb = cpool.tile([128, CIT * BL], bf16)
            s1_sb = cpool.tile([128, CIT * BL], bf16)
            s_sb = cpool.tile([128, CIT * BL], f32)
            mlp_layer(
                lambda kt: pp1_sb[:, _PY + kt * BL : _PY + (kt + 1) * BL],
                4, pp1_sb, _PW0, 0, s0_sb,
            )
            mlp_layer(
                lambda kt: s0_sb[:, kt * BL : (kt + 1) * BL],
                2, pp2_sb, 0, CIT, s1_sb,
            )
            mlp_layer(
                lambda kt: s1_sb[:, kt * BL : (kt + 1) * BL],
                2, pp3_sb, 0, 2 * CIT, s_sb,
            )

            # ---- modulated transformed weights: Ub = U0 * s[b, ci] (fp16) ----
            ub = {}

            def fold_ub(b, ci_t, co_t):
                t = cpool.tile([128, UBF], f16, tag=f"ub{b}{ci_t}{co_t}")
                nc.vector.tensor_scalar_mul(
                    t[:], u0_sbs[(ci_t, co_t)][:],
                    s_sb[:, ci_t * BL + b : ci_t * BL + b + 1],
                )
                ub[(b, ci_t, co_t)] = t

            for co_t in range(COT):
                for ci_t in range(CIT):
                    fold_ub(0, ci_t, co_t)
            # b=1 folds happen after the first V builds (DVE is startup-critical)

            # ---- V = B^T d row-transform per (sample, ci-tile) ----
            # Scale steps run on the scalar engine; adds/subs on DVE (fp16 2x).
            vs = {}

            def build_v(b, ci_t, i0, icnt):
                g = grids[(b, ci_t)]
                r0 = 4 * i0
                rend = r0 + 4 * icnt - 3

                def d(p):
                    return g[:, r0 + p : rend + p : 4, :]   # [128, icnt, 66]

                if (b, ci_t) not in vs:
                    v = vpool.tile(
                        [128, UD * IT * GW], f16, name=f"v{b}{ci_t}", tag="v"
                    )
                    vs[(b, ci_t)] = v[:].rearrange("p (u i w) -> p u i w", i=IT, w=GW)
                vv = vs[(b, ci_t)]

                def V(u):
                    return vv[:, u, i0 : i0 + icnt, :]

                def tmp(tag):
                    t = tpool.tile(
                        [128, icnt * GW], f16,
                        name=f"{tag}_{b}{ci_t}{i0}", tag=tag,
                    )
                    return t[:].rearrange("p (i w) -> p i w", w=GW)

                t1, t2, t3, t4 = tmp("t1"), tmp("t2"), tmp("t3"), tmp("t4")
                t6, t7, t8 = tmp("t6"), tmp("t7"), tmp("t8")
                q1, q2, q3 = tmp("q1"), tmp("q2"), tmp("q3")
                tt = nc.vector.tensor_tensor
                tt(t1, d(1), d(2), ADD)
                tt(t3, d(1), d(2), SUB)
                nc.scalar.mul(q1, t1, 4.0)
                tt(t2, d(3), d(4), ADD)
                tt(V(1), q1, t2, SUB)              # V1 = 4(d1+d2)-(d3+d4) = -w1
                nc.scalar.mul(q2, t3, 4.0)
                tt(t4, d(3), d(4), SUB)
                tt(V(2), q2, t4, SUB)              # V2 = 4(d1-d2)-(d3-d4)
                tt(t6, d(3), d(1), SUB)
                nc.scalar.mul(q3, t6, 2.0)
                tt(t7, d(4), d(2), SUB)
                tt(V(3), q3, t7, ADD)              # V3 = 2(d3-d1)+(d4-d2)
                tt(V(4), t7, q3, SUB)              # V4 = (d4-d2)-2(d3-d1)
                nc.scalar.mul(q1, d(0), 4.0)
                nc.scalar.mul(q2, d(2), 5.0)
                tt(t8, q1, q2, SUB)
                tt(V(0), t8, d(4), ADD)            # V0 = 4d0-5d2+d4
                nc.scalar.mul(q1, d(1), 4.0)
                nc.scalar.mul(q2, d(3), 5.0)
                tt(t8, q1, q2, SUB)
                tt(V(5), t8, d(5), ADD)            # V5 = 4d1-5d3+d5

            build_v(0, 0, 0, CHI)
            build_v(0, 1, 0, CHI)
            build_v(0, 0, CHI, CHI)
            build_v(0, 1, CHI, CHI)
            for co_t in range(COT):
                for ci_t in range(CIT):
                    fold_ub(1, ci_t, co_t)

            # ---- conv + inverse transform per (sample, co_t, i-tile chunk) ----
            def conv_chunk(b, co_t, i0, icnt, ci_major=False, uorder=(1, 2, 3, 4, 0, 5)):
                n = icnt * W
                ps = [
                    cpsum.tile([128, n], f32, name=f"ps_{b}_{co_t}_{i0}_{u}", tag=f"ps")
                    for u in range(UD)
                ]

                def mm(u, ci_t, kw):
                    nc.tensor.matmul(
                        ps[u][:],
                        ub[(b, ci_t, co_t)][:, (u * K + kw) * 128 :][:, :128],
                        vs[(b, ci_t)][:, u, i0 : i0 + icnt, kw : kw + W],
                        start=(ci_t == 0 and kw == 0),
                        stop=(ci_t == CIT - 1 and kw == K - 1),
                    )

                if ci_major:
                    # first chunk: don't gate the whole chunk on the second V
                    for ci_t in range(CIT):
                        for kw in range(K):
                            for u in uorder:
                                mm(u, ci_t, kw)
                else:
                    # u-major: each P[u] completes after 6 matmuls so drains
                    # and the inverse pipeline alongside the later matmuls
                    for u in uorder:
                        for ci_t in range(CIT):
                            for kw in range(K):
                                mm(u, ci_t, kw)

                def mtmp(tag):
                    t = mpool.tile(
                        [128, CHI * W], f16, name=f"{tag}_{b}_{co_t}_{i0}", tag=tag
                    )
                    return t[:][:, :n]

                # m1/m2 drain on Act; r/s come straight out of PSUM on gpsimd
                # (saving the m3/m4 drains); m0/m5 drain on gpsimd. U row 1 and
                # device V1 are both sign-flipped, so the PSUM values are the
                # true Winograd products and A^T applies directly.
                # High priority: drains free PSUM banks and must preempt
                # queued V-transform work on every engine.
                ctx = tc.high_priority(offset=10000)
                ctx.__enter__()
                m1, m2, m0, m5 = mtmp("m1"), mtmp("m2"), mtmp("m0"), mtmp("m5")
                p_, q_, r_, s_ = mtmp("ip"), mtmp("iq"), mtmp("ir"), mtmp("is")
                s2, r4, s8 = mtmp("s2"), mtmp("r4"), mtmp("s8")
                oa, ob = m1, m2          # m1/m2 are dead once p/q are formed
                tt = nc.vector.tensor_tensor
                gt = nc.gpsimd.tensor_tensor
                nc.scalar.copy(m1, ps[1][:])
                nc.scalar.copy(m2, ps[2][:])
                gt(r_, ps[3][:], ps[4][:], ADD)
                gt(s_, ps[3][:], ps[4][:], SUB)
                nc.gpsimd.tensor_copy(m0, ps[0][:])
                nc.gpsimd.tensor_copy(m5, ps[5][:])
                gt(p_, m1, m2, ADD)                # p = m1 + m2
                gt(q_, m1, m2, SUB)                # q = m1 - m2
                nc.scalar.mul(s2, s_, 2.0)
                nc.scalar.mul(r4, r_, 4.0)
                nc.scalar.mul(s8, s_, 8.0)
                o = opool.tile(
                    [128, 4 * CHI * W], f16, name=f"o_{b}_{co_t}_{i0}", tag="o"
                )
                o = o[:, : 4 * n]
                ov = o[:].rearrange("p (h w) -> p h w", w=W)
                nr = 4 * icnt
                tt(oa, m0, p_, ADD)
                tt(ov[:, 0 : nr - 3 : 4, :], oa, r_, ADD)       # o0 = m0+p+r
                tt(ov[:, 1 : nr - 2 : 4, :], q_, s2, ADD)       # o1 = q+2s
                tt(ov[:, 2 : nr - 1 : 4, :], p_, r4, ADD)       # o2 = p+4r
                tt(ob, q_, m5, ADD)
                tt(ov[:, 3 : nr : 4, :], ob, s8, ADD)           # o3 = q+8s+m5
                nc.sync.dma_start(
                    out_ext[b, co_t][:, 4 * W * i0 : 4 * W * (i0 + icnt)], o[:]
                )
                ctx.__exit__(None, None, None)

            # interleave the second sample's V builds between chunks so no
            # engine queue ever parks a PSUM-freeing drain behind them
            conv_chunk(0, 0, 0, CHI, ci_major=True)
            conv_chunk(0, 0, CHI, CHI)
            load_grid(1, 0)
            conv_chunk(0, 1, 0, CHI)
            build_v(1, 0, 0, CHI)
            load_grid(1, 1)
            conv_chunk(0, 1, CHI, CHI)
            build_v(1, 1, 0, CHI)
            conv_chunk(1, 0, 0, CHI)
            build_v(1, 0, CHI, CHI)
            conv_chunk(1, 0, CHI, CHI)
            build_v(1, 1, CHI, CHI)
            conv_chunk(1, 1, 0, CHI)
            # split the final chunk so its drain/inverse/store tail overlaps
            # the previous half's matmuls
            conv_chunk(1, 1, CHI, CHI // 2, uorder=(3, 4, 1, 2, 5, 0))
            conv_chunk(1, 1, CHI + CHI // 2, CHI // 2, uorder=(3, 4, 1, 2, 5, 0))

    nc.compile()
    return nc


def _get_nc():
    global _COMPILED
    if _COMPILED is None:
        _COMPILED = _build()
    return _COMPILED


def _prep_in_maps(x, y, w0, b0, w1, b1, w2, b2, conv_w):
    import ml_dtypes

    BF = ml_dtypes.bfloat16
    x = np.ascontiguousarray(x, dtype=np.float32)
    y = np.ascontiguousarray(y, dtype=np.float32)

    # packed per-core-invariant MLP params (bf16 weights, fp32 biases)
    pp1_shared = np.empty((128, _P1TOT), dtype=BF)
    pp1_shared[:, _PW0 : _PW0 + 4 * C_IN] = (
        w0.astype(np.float32).T.reshape(4, 128, C_IN).transpose(1, 0, 2).reshape(128, 4 * C_IN)
    ).astype(BF)
    pp2 = np.ascontiguousarray(
        w1.astype(np.float32).T.reshape(2, 128, C_IN).transpose(1, 0, 2).reshape(128, 2 * C_IN)
    ).astype(BF)
    pp3 = np.ascontiguousarray(
        w2.astype(np.float32).T.reshape(2, 128, C_IN).transpose(1, 0, 2).reshape(128, 2 * C_IN)
    ).astype(BF)
    bias = np.empty((128, _NBIAS), dtype=np.float32)
    for i, bb in enumerate((b0, b1, b2)):
        bias[:, i * CIT : (i + 1) * CIT] = bb.astype(np.float32).reshape(CIT, 128).T

    # U0 = G @ conv_w along kh: (O,I,kh,kw) -> (I_t, ci, co_t, u, kw, co)
    T = np.einsum("uh,oihw->oiuw", _G, conv_w.astype(np.float64))
    u0 = np.ascontiguousarray(
        T.transpose(1, 2, 3, 0)
        .reshape(CIT, 128, UD, K, COT, 128)
        .transpose(0, 4, 1, 2, 3, 5)
        .reshape(CIT, COT, 128, UBF)
    ).astype(np.float16)

    xg_all = np.zeros((B, CIT, 128, GH, GW), dtype=np.float16)
    xg_all[:, :, :, 1 : H + 1, 1 : W + 1] = x.reshape(B, CIT, 128, H, W).astype(np.float16)
    xg_all = xg_all.reshape(B, CIT, 128, GH * GW)

    in_maps = []
    for c in range(NCORES):
        sl = slice(c * BL, (c + 1) * BL)
        pp1 = pp1_shared.copy()
        pp1[:, _PY : _PY + 4 * BL] = (
            y[sl].T.reshape(4, 128, BL).transpose(1, 0, 2).reshape(128, 4 * BL)
        ).astype(BF)
        in_maps.append(
            {
                "pp1": pp1,
                "pp2": pp2,
                "pp3": pp3,
                "bias": bias,
                "u0": u0,
                "xg": np.ascontiguousarray(xg_all[sl]),
            }
        )
    return in_maps


def _run(in_maps, trace=False):
    from concourse.bass_utils import run_bass_kernel_spmd

    nc = _get_nc()
    res = run_bass_kernel_spmd(nc, in_maps, list(range(NCORES)), trace=trace)
    out = np.concatenate(
        [
            res.results[c]["out"].astype(np.float32).reshape(BL, C_OUT, H, W)
            for c in range(NCORES)
        ],
        axis=0,
    )
    return out, res


def kernel(x, y, w0, b0, w1, b1, w2, b2, conv_w):
    in_maps = _prep_in_maps(x, y, w0, b0, w1, b1, w2, b2, conv_w)
    out, _ = _run(in_maps, trace=False)
    return out


# revision 18
# speedup vs baseline: 1.2645x; 1.2645x over previous
"""Trainium2 Bass kernel for per-sample channel-modulated 3x3 conv (CoModConv).

Math (matches the reference nn.Module):
    s = lrelu(lrelu(lrelu(y @ w0.T + b0) @ w1.T + b1) @ w2.T + b2)   # (B, C_in)
    out = conv3x3(x * s[:, :, None, None], conv_w, pad=1)            # (B, C_out, H, W)

Strategy: data-parallel over batch, 2 samples per NeuronCore (8 cores), with a
1-D Winograd F(4,3) transform along H (direct 3-tap conv along W), all in fp16:
  - Host precomputes U0 = G @ conv_w (transform of the shared weight along kh);
    the per-sample channel scale s folds into U on device (one tensor_scalar
    per (sample, ci-tile)), so modulation is free.
  - Device builds V = B^T d (6 transformed row-planes per input tile of 4 rows)
    on the vector engine (fp16, 2x mode), with the constant-scale steps on the
    scalar engine.
  - The conv becomes, per (sample, co_t, 8-i-tile chunk), 36 accumulating
    128x128x512 fp16 matmuls (ci_t x kw x u) into 6 PSUM banks -- 288 matmuls
    per core vs 576 for direct conv (1.5x fewer after transform overhead;
    2.25x fewer MACs land in 6/9 of the direct kernel taps).
  - Inverse transform A^T m (6 -> 4 output rows) runs on scalar-engine drains
    (PSUM fp32 -> fp16) + 13 small vector ops per chunk, writing interleaved
    output rows; outputs DMA out in fp16 and are upcast on host.
fp16 (not bf16) keeps the Winograd transform numerics comfortably inside the
tolerance: measured rel err ~4.6e-3 vs ~4.3e-2 for bf16.
"""

import numpy as np

B, D_CAT, C_IN, C_OUT, K, H, W = 16, 512, 256, 256, 3, 64, 64
NCORES = 8
BL = B // NCORES          # samples per core (2)
CIT = C_IN // 128         # ci tiles (2)
COT = C_OUT // 128        # co tiles (2)
GH = H + 2                # padded grid rows (66)
GW = W + 2                # padded grid cols (66)
UD = 6                    # F(4,3) transform length
IT = 16                   # winograd i-tiles along H (4 output rows each)
CHI = 8                   # i-tiles per PSUM chunk (8*64 = 512 cols)
NCH = IT // CHI           # chunks per (sample, co_t) (2)
UBF = UD * K * 128        # Ub columns per co_t block (2304)

# packed MLP-param column offsets (per partition). Weights and y ship in bf16
# (pp1 = y + w0, pp2 = w1, pp3 = w2, ordered by first use); biases in fp32.
_PY = 0                       # y^T:   4 k-tiles x BL
_PW0 = _PY + 4 * BL           # w0^T:  4 k-tiles x 256
_P1TOT = _PW0 + 4 * C_IN
_P2TOT = 2 * C_IN             # w1^T
_P3TOT = 2 * C_IN             # w2^T
_NBIAS = 3 * CIT              # b0, b1, b2 per ci-tile (fp32)

# F(4,3) weight transform (G), with the sign of row u=1 folded in because the
# device computes V[1] = 4(d1+d2) - (d3+d4) = -B^T[1] d.
_G = np.array(
    [
        [1 / 4, 0, 0],
        [-1 / 6, -1 / 6, -1 / 6],
        [-1 / 6, 1 / 6, -1 / 6],
        [1 / 24, 1 / 12, 1 / 6],
        [1 / 24, -1 / 12, 1 / 6],
        [0, 0, 1],
    ],
    dtype=np.float64,
) * np.array([1, -1, 1, 1, -1, 1], dtype=np.float64)[:, None]

_COMPILED = None


def _build():
    import concourse.mybir as mybir
    import concourse.tile as tile
    from concourse import bacc

    bf16 = mybir.dt.bfloat16
    f16 = mybir.dt.float16
    f32 = mybir.dt.float32
    Prelu = mybir.ActivationFunctionType.Prelu
    ADD = mybir.AluOpType.add
    SUB = mybir.AluOpType.subtract
    MUL = mybir.AluOpType.mult

    nc = bacc.Bacc("TRN2", target_bir_lowering=False, debug=False, num_devices=NCORES)

    pp1_in = nc.declare_dram_parameter("pp1", [128, _P1TOT], bf16, isOutput=False)
    pp2_in = nc.declare_dram_parameter("pp2", [128, _P2TOT], bf16, isOutput=False)
    pp3_in = nc.declare_dram_parameter("pp3", [128, _P3TOT], bf16, isOutput=False)
    bias_in = nc.declare_dram_parameter("bias", [128, _NBIAS], f32, isOutput=False)
    u0_in = nc.declare_dram_parameter("u0", [CIT, COT, 128, UBF], f16, isOutput=False)
    xg_in = nc.declare_dram_parameter("xg", [BL, CIT, 128, GH * GW], f16, isOutput=False)
    out_ext = nc.declare_dram_parameter("out", [BL, COT, 128, H * W], f16, isOutput=True)

    with tile.TileContext(nc) as tc:
        with (
            tc.tile_pool(name="const", bufs=1) as cpool,
            tc.tile_pool(name="grid", bufs=2) as gpool,
            tc.tile_pool(name="vpool", bufs=4) as vpool,
            tc.tile_pool(name="tmp", bufs=2) as tpool,
            tc.tile_pool(name="u0p", bufs=2) as upool,
            tc.tile_pool(name="minv", bufs=2) as mpool,
            tc.tile_pool(name="oout", bufs=2) as opool,
            tc.tile_pool(name="cpsum", bufs=8, space="PSUM") as cpsum,
        ):
            MAX = mybir.AluOpType.max
            grids = {}
            u0_sbs = {}

            GSPL = (4 * CHI + 2) * GW          # rows 0-33 cover the first V half

            def load_grid(b, ci_t):
                t = gpool.tile([128, GH * GW], f16, name=f"g{b}{ci_t}", tag="g")
                nc.gpsimd.dma_start(t[:, :GSPL], xg_in[b, ci_t][:, :GSPL])
                nc.gpsimd.dma_start(t[:, GSPL:], xg_in[b, ci_t][:, GSPL:])
                grids[(b, ci_t)] = t[:].rearrange("p (h w) -> p h w", w=GW)

            def load_u0(ci_t, co_t):
                t = upool.tile([128, UBF], f16, name=f"u0_{ci_t}{co_t}", tag="u0")
                nc.gpsimd.dma_start(t[:], u0_in[ci_t, co_t])
                u0_sbs[(ci_t, co_t)] = t

            # ---- DMAs, ordered so the first-chunk dependencies land first.
            # x grids and U0 go via the gpsimd SWDGE queue, params via HWDGE;
            # the sim serializes all DMA on one device, so order IS latency.
            t = gpool.tile([128, GH * GW], f16, name="g00", tag="g")
            nc.gpsimd.dma_start(t[:, :GSPL], xg_in[0, 0][:, :GSPL])
            grids[(0, 0)] = t[:].rearrange("p (h w) -> p h w", w=GW)
            g00 = t
            pp1_sb = cpool.tile([128, _P1TOT], bf16)
            nc.sync.dma_start(pp1_sb[:], pp1_in[:])
            t2_ = gpool.tile([128, GH * GW], f16, name="g01", tag="g")
            nc.gpsimd.dma_start(t2_[:, :GSPL], xg_in[0, 1][:, :GSPL])
            grids[(0, 1)] = t2_[:].rearrange("p (h w) -> p h w", w=GW)
            g01 = t2_
            bias_sb = cpool.tile([128, _NBIAS], f32)
            nc.sync.dma_start(bias_sb[:], bias_in[:])
            pp2_sb = cpool.tile([128, _P2TOT], bf16)
            nc.sync.dma_start(pp2_sb[:], pp2_in[:])
            pp3_sb = cpool.tile([128, _P3TOT], bf16)
            nc.sync.dma_start(pp3_sb[:], pp3_in[:])
            load_u0(0, 0)
            load_u0(1, 0)
            nc.gpsimd.dma_start(g00[:, GSPL:], xg_in[0, 0][:, GSPL:])
            nc.gpsimd.dma_start(g01[:, GSPL:], xg_in[0, 1][:, GSPL:])
            load_u0(0, 1)
            load_u0(1, 1)

            # ---- style MLP. Matmuls on PE; bias + leaky-relu run on the
            # (otherwise idle) gpsimd engine so the scalar engine's in-order
            # queue stays free for V-transform scale ops. ----
            def mlp_layer(rhs_of_kt, kts, w_sb, w_base, bias_col, out_sb):
                for ct in range(CIT):
                    mps = cpsum.tile([128, 512], f32, name=f"mlp_{w_base}_{ct}", tag="ps")
                    for kt in range(kts):
                        nc.tensor.matmul(
                            mps[:, :BL],
                            w_sb[:, w_base + kt * C_IN + ct * 128 :][:, :128],
                            rhs_of_kt(kt),
                            start=(kt == 0),
                            stop=(kt == kts - 1),
                        )
                    za = tpool.tile([128, BL], f32, name=f"za_{w_base}_{ct}", tag="za")
                    zl = tpool.tile([128, BL], f32, name=f"zl_{w_base}_{ct}", tag="zl")
                    nc.gpsimd.tensor_scalar_add(
                        za[:], mps[:, :BL], bias_sb[:, bias_col + ct : bias_col + ct + 1]
                    )
                    nc.gpsimd.tensor_scalar_mul(zl[:], za[:], 0.01)
                    nc.gpsimd.tensor_tensor(
                        out_sb[:, ct * BL : (ct + 1) * BL], za[:], zl[:], MAX
                    )

            s0_sb = cpool.tile([128, CIT * BL], bf16)
            s1_sb = cpool.tile([128, CIT * BL], bf16)
            s_sb = cpool.tile([128, CIT * BL], f32)
            mlp_layer(
                lambda kt: pp1_sb[:, _PY + kt * BL : _PY + (kt + 1) * BL],
                4, pp1_sb, _PW0, 0, s0_sb,
            )
            mlp_layer(
                lambda kt: s0_sb[:, kt * BL : (kt + 1) * BL],
                2, pp2_sb, 0, CIT, s1_sb,
            )
            mlp_layer(
                lambda kt: s1_sb[:, kt * BL : (kt + 1) * BL],
                2, pp3_sb, 0, 2 * CIT, s_sb,
            )

            # ---- modulated transformed weights: Ub = U0 * s[b, ci] (fp16) ----
            ub = {}

            def fold_ub(b, ci_t, co_t):
                t = cpool.tile([128, UBF], f16, tag=f"ub{b}{ci_t}{co_t}")
                nc.vector.tensor_scalar_mul(
                    t[:], u0_sbs[(ci_t, co_t)][:],
                    s_sb[:, ci_t * BL + b : ci_t * BL + b + 1],
                )
                ub[(b, ci_t, co_t)] = t

            for co_t in range(COT):
                for ci_t in range(CIT):
                    fold_ub(0, ci_t, co_t)
            # b=1 folds happen after the first V builds (DVE is startup-critical)

            # ---- V = B^T d row-transform per (sample, ci-tile) ----
            # Scale steps run on the scalar engine; adds/subs on DVE (fp16 2x).
            vs = {}

            def build_v(b, ci_t, i0, icnt):
                g = grids[(b, ci_t)]
                r0 = 4 * i0
                rend = r0 + 4 * icnt - 3

                def d(p):
                    return g[:, r0 + p : rend + p : 4, :]   # [128, icnt, 66]

                if (b, ci_t) not in vs:
                    v = vpool.tile(
                        [128, UD * IT * GW], f16, name=f"v{b}{ci_t}", tag="v"
                    )
                    vs[(b, ci_t)] = v[:].rearrange("p (u i w) -> p u i w", i=IT, w=GW)
                vv = vs[(b, ci_t)]

                def V(u):
                    return vv[:, u, i0 : i0 + icnt, :]

                def tmp(tag):
                    t = tpool.tile(
                        [128, icnt * GW], f16,
                        name=f"{tag}_{b}{ci_t}{i0}", tag=tag,
                    )
                    return t[:].rearrange("p (i w) -> p i w", w=GW)

                t1, t2, t3, t4 = tmp("t1"), tmp("t2"), tmp("t3"), tmp("t4")
                t6, t7, t8 = tmp("t6"), tmp("t7"), tmp("t8")
                q1, q2 = tmp("q1"), tmp("q2")
                tt = nc.vector.tensor_tensor
                tt(t1, d(1), d(2), ADD)
                tt(t3, d(1), d(2), SUB)
                nc.scalar.mul(q1, t1, 4.0)
                tt(t2, d(3), d(4), ADD)
                tt(V(1), q1, t2, SUB)              # V1 = 4(d1+d2)-(d3+d4) = -w1
                nc.scalar.mul(q2, t3, 4.0)
                tt(t4, d(3), d(4), SUB)
                tt(V(2), q2, t4, SUB)              # V2 = 4(d1-d2)-(d3-d4)
                tt(t6, d(3), d(1), SUB)
                tt(t7, d(4), d(2), SUB)
                nc.gpsimd.scalar_tensor_tensor(
                    V(3), t6, 2.0, t7, MUL, ADD
                )                                  # V3 = 2(d3-d1)+(d4-d2)
                nc.gpsimd.scalar_tensor_tensor(
                    V(4), t6, 2.0, t7, MUL, SUB
                )                                  # -V4 = 2(d3-d1)-(d4-d2); sign in U
                nc.scalar.mul(q1, d(0), 4.0)
                nc.scalar.mul(q2, d(2), 5.0)
                tt(t8, q1, q2, SUB)
                tt(V(0), t8, d(4), ADD)            # V0 = 4d0-5d2+d4
                nc.scalar.mul(q1, d(1), 4.0)
                nc.scalar.mul(q2, d(3), 5.0)
                tt(t8, q1, q2, SUB)
                tt(V(5), t8, d(5), ADD)            # V5 = 4d1-5d3+d5

            build_v(0, 0, 0, CHI)
            build_v(0, 1, 0, CHI)
            for co_t in range(COT):
                for ci_t in range(CIT):
                    fold_ub(1, ci_t, co_t)

            # ---- conv + inverse transform per (sample, co_t, i-tile chunk) ----
            def conv_chunk(b, co_t, i0, icnt, ci_major=False, uorder=(1, 2, 3, 4, 0, 5)):
                n = icnt * W
                ps = [
                    cpsum.tile([128, n], f32, name=f"ps_{b}_{co_t}_{i0}_{u}", tag=f"ps")
                    for u in range(UD)
                ]

                def mm(u, ci_t, kw):
                    nc.tensor.matmul(
                        ps[u][:],
                        ub[(b, ci_t, co_t)][:, (u * K + kw) * 128 :][:, :128],
                        vs[(b, ci_t)][:, u, i0 : i0 + icnt, kw : kw + W],
                        start=(ci_t == 0 and kw == 0),
                        stop=(ci_t == CIT - 1 and kw == K - 1),
                    )

                if ci_major:
                    # startup chunks: u-outer within each ci so each u-group's
                    # matmuls begin as soon as that single V plane is built
                    for ci_t in range(CIT):
                        for u in uorder:
                            for kw in range(K):
                                mm(u, ci_t, kw)
                else:
                    # u-major: each P[u] completes after 6 matmuls so drains
                    # and the inverse pipeline alongside the later matmuls
                    for u in uorder:
                        for ci_t in range(CIT):
                            for kw in range(K):
                                mm(u, ci_t, kw)

                def mtmp(tag):
                    t = mpool.tile(
                        [128, CHI * W], f16, name=f"{tag}_{b}_{co_t}_{i0}", tag=tag
                    )
                    return t[:][:, :n]

                # m1/m2 drain on Act; r/s come straight out of PSUM on gpsimd
                # (saving the m3/m4 drains); m0/m5 drain on gpsimd. U row 1 and
                # device V1 are both sign-flipped, so the PSUM values are the
                # true Winograd products and A^T applies directly.
                # High priority: drains free PSUM banks and must preempt
                # queued V-transform work on every engine.
                ctx = tc.high_priority(offset=10000)
                ctx.__enter__()
                m0, m5 = mtmp("m0"), mtmp("m5")
                p_, q_, r_, s_ = mtmp("ip"), mtmp("iq"), mtmp("ir"), mtmp("is")
                s2, r4, s8 = mtmp("s2"), mtmp("r4"), mtmp("s8")
                oa, ob = mtmp("oa"), mtmp("ob")
                tt = nc.vector.tensor_tensor
                gt = nc.gpsimd.tensor_tensor
                tsm = nc.vector.tensor_scalar_mul
                gt(p_, ps[1][:], ps[2][:], ADD)    # p = m1 + m2
                gt(q_, ps[1][:], ps[2][:], SUB)    # q = m1 - m2
                gt(r_, ps[3][:], ps[4][:], ADD)
                gt(s_, ps[3][:], ps[4][:], SUB)
                nc.scalar.copy(m0, ps[0][:])
                nc.scalar.copy(m5, ps[5][:])
                tsm(s2, s_, 2.0)
                tsm(r4, r_, 4.0)
                tsm(s8, s_, 8.0)
                o = opool.tile(
                    [128, 4 * CHI * W], f16, name=f"o_{b}_{co_t}_{i0}", tag="o"
                )
                o = o[:, : 4 * n]
                ov = o[:].rearrange("p (h w) -> p h w", w=W)
                nr = 4 * icnt
                tt(oa, m0, p_, ADD)
                tt(ov[:, 0 : nr - 3 : 4, :], oa, r_, ADD)       # o0 = m0+p+r
                tt(ov[:, 1 : nr - 2 : 4, :], q_, s2, ADD)       # o1 = q+2s
                tt(ov[:, 2 : nr - 1 : 4, :], p_, r4, ADD)       # o2 = p+4r
                tt(ob, q_, m5, ADD)
                tt(ov[:, 3 : nr : 4, :], ob, s8, ADD)           # o3 = q+8s+m5
                nc.sync.dma_start(
                    out_ext[b, co_t][:, 4 * W * i0 : 4 * W * (i0 + icnt)], o[:]
                )
                ctx.__exit__(None, None, None)

            # Each chunk's emission is paired with the V half-build that the
            # chunk AFTER next consumes, so every engine queue always holds
            # (ready chunk-drain work) before (future V-build work) and PSUM
            # banks free promptly. ci_major chunks tolerate a late second V.
            conv_chunk(0, 0, 0, CHI, ci_major=True)
            build_v(0, 0, CHI, CHI)
            conv_chunk(0, 1, 0, CHI)
            build_v(0, 1, CHI, CHI)
            load_grid(1, 0)
            conv_chunk(0, 0, CHI, CHI, ci_major=True)
            build_v(1, 0, 0, CHI)
            load_grid(1, 1)
            conv_chunk(0, 1, CHI, CHI)
            build_v(1, 1, 0, CHI)
            conv_chunk(1, 0, 0, CHI, ci_major=True)
            build_v(1, 0, CHI, CHI)
            conv_chunk(1, 1, 0, CHI)
            build_v(1, 1, CHI, CHI)
            conv_chunk(1, 0, CHI, CHI, ci_major=True)
            # taper the final chunks so the drain/inverse/store tail of each
            # overlaps the next one's matmuls
            conv_chunk(1, 1, CHI, CHI // 2, uorder=(3, 4, 1, 2, 5, 0))
            conv_chunk(1, 1, 12, 2, uorder=(3, 4, 1, 2, 5, 0))
            conv_chunk(1, 1, 14, 2, uorder=(3, 4, 1, 2, 5, 0))

    nc.compile()
    return nc


def _get_nc():
    global _COMPILED
    if _COMPILED is None:
        _COMPILED = _build()
    return _COMPILED


def _prep_in_maps(x, y, w0, b0, w1, b1, w2, b2, conv_w):
    import ml_dtypes

    BF = ml_dtypes.bfloat16
    x = np.ascontiguousarray(x, dtype=np.float32)
    y = np.ascontiguousarray(y, dtype=np.float32)

    # packed per-core-invariant MLP params (bf16 weights, fp32 biases)
    pp1_shared = np.empty((128, _P1TOT), dtype=BF)
    pp1_shared[:, _PW0 : _PW0 + 4 * C_IN] = (
        w0.astype(np.float32).T.reshape(4, 128, C_IN).transpose(1, 0, 2).reshape(128, 4 * C_IN)
    ).astype(BF)
    pp2 = np.ascontiguousarray(
        w1.astype(np.float32).T.reshape(2, 128, C_IN).transpose(1, 0, 2).reshape(128, 2 * C_IN)
    ).astype(BF)
    pp3 = np.ascontiguousarray(
        w2.astype(np.float32).T.reshape(2, 128, C_IN).transpose(1, 0, 2).reshape(128, 2 * C_IN)
    ).astype(BF)
    bias = np.empty((128, _NBIAS), dtype=np.float32)
    for i, bb in enumerate((b0, b1, b2)):
        bias[:, i * CIT : (i + 1) * CIT] = bb.astype(np.float32).reshape(CIT, 128).T

    # U0 = G @ conv_w along kh: (O,I,kh,kw) -> (I_t, ci, co_t, u, kw, co)
    T = np.einsum("uh,oihw->oiuw", _G, conv_w.astype(np.float64))
    u0 = np.ascontiguousarray(
        T.transpose(1, 2, 3, 0)
        .reshape(CIT, 128, UD, K, COT, 128)
        .transpose(0, 4, 1, 2, 3, 5)
        .reshape(CIT, COT, 128, UBF)
    ).astype(np.float16)

    xg_all = np.zeros((B, CIT, 128, GH, GW), dtype=np.float16)
    xg_all[:, :, :, 1 : H + 1, 1 : W + 1] = x.reshape(B, CIT, 128, H, W).astype(np.float16)
    xg_all = xg_all.reshape(B, CIT, 128, GH * GW)

    in_maps = []
    for c in range(NCORES):
        sl = slice(c * BL, (c + 1) * BL)
        pp1 = pp1_shared.copy()
        pp1[:, _PY : _PY + 4 * BL] = (
            y[sl].T.reshape(4, 128, BL).transpose(1, 0, 2).reshape(128, 4 * BL)
        ).astype(BF)
        in_maps.append(
            {
                "pp1": pp1,
                "pp2": pp2,
                "pp3": pp3,
                "bias": bias,
                "u0": u0,
                "xg": np.ascontiguousarray(xg_all[sl]),
            }
        )
    return in_maps


def _run(in_maps, trace=False):
    from concourse.bass_utils import run_bass_kernel_spmd

    nc = _get_nc()
    res = run_bass_kernel_spmd(nc, in_maps, list(range(NCORES)), trace=trace)
    out = np.concatenate(
        [
            res.results[c]["out"].astype(np.float32).reshape(BL, C_OUT, H, W)
            for c in range(NCORES)
        ],
        axis=0,
    )
    return out, res


def kernel(x, y, w0, b0, w1, b1, w2, b2, conv_w):
    in_maps = _prep_in_maps(x, y, w0, b0, w1, b1, w2, b2, conv_w)
    out, _ = _run(in_maps, trace=False)
    return out
